# revision 38
# baseline (speedup 1.0000x reference)
"""CRF loss kernel for Trainium2 (8 NeuronCores, data-parallel over batch).

Problem (hardcoded shapes): scores [B=128, T=256, K=64, K=64] f32,
targets [128, 256] int (flattened from_tag*K + to_tag), lengths [128] int.

loss = (sum_b fs[b, END] - gold) / B  where fs is the CRF forward
(log-domain) scan and gold is the gathered gold-path score.

Strategy (per core, 16 batch rows; ~242us HW vs the 845us f32 baseline):
  * Linear-domain forward scan with a constant per-step 2^-7 scale that
    is folded into the scores host-side (sc' = sc - 7*ln2), so the
    device step is a pure bf16 matmul + PSUM->SBUF cast:
        a_t = E'_t^T a_{t-1},   E'_t = exp(sc'_t)
  * Padded timesteps (t >= L_b) are overwritten host-side with an
    "identity slab" (0 on the diagonal, -100 off it), so exp() of them
    is exactly the identity matrix and the scan needs no per-row
    freezing, masking, or per-step state dumps: a_{T-1} == a_{L_b-1}
    automatically and only the final state is read back.
  * Host pre-interleaves the scores to [b][blk][kf][t_in_blk][kto]
    (bf16), so every DMA descriptor is a contiguous 2 KiB line; strip
    descriptor generation is split across the sync and gpsimd queues.
  * Each batch-row pair does two [64,64]x[64,1] bf16 matmuls per step:
    the even row at PE tile position (0,0), the odd row at (64,64)
    (derived from the partition offsets), so the next state lands
    PACKED in PSUM [128, 1] and ONE plain DVE copy per group per step
    casts it back to SBUF.  The 8 pairs form 3 groups (3/3/2) with
    independent PSUM-bank/state tiles; interleaving their strip/exp
    emission keeps the tile scheduler from serializing the chains, and
    the two spare group bursts hide each group's copy+semaphore chain.
  * gold: indirect DMA element-gather from the raw f32 scores (the
    2-byte gather path collapses per-partition indirection), masked by
    a huge sentinel index; the free-axis reduce + 128->1 matmul-reduce
    run after the scan (an early DVE reduce would head-block the
    in-order DVE queue behind the gather).
  * a0/afin travel transposed [pair, 128] and are transposed on-chip
    via PE identity matmuls: a [128, x] bf16 DRAM tensor would shatter
    into per-partition 4-byte packets whose completion semaphores
    trickle for ~10us in the epilogue.
  * Host finishes per row: fs_b = log(a_fin[END]) + L_b * 7*ln2.
"""

import math

import ml_dtypes
import numpy as np

import concourse.bacc as bacc
import concourse.bass as bass
import concourse.tile as tile
from concourse import mybir
from concourse.bass_utils import run_bass_kernel_spmd

F32 = mybir.dt.float32
BF16 = mybir.dt.bfloat16
I32 = mybir.dt.int32

B = 128
T = 256
K = 64
START = 62
END = 63
NCORES = 8
BL = B // NCORES          # 16 local batch rows per core
NPAIR = BL // 2           # 8
GROUPS = [[0, 1, 2], [3, 4, 5], [6, 7]]  # pipeline groups (pair ids)
NGRP = len(GROUPS)
W = 16                    # timesteps per strip
NBLK = T // W             # 16
G = BL * T // 128         # gold gather indices per partition (32)
LOG_C = 7.0 * math.log(2.0)  # per-step scale 2^-7, folded into scores
PAD_OFFDIAG = -100.0      # exp() == 0 in bf16
SENTINEL = 0x7FFFFF00     # OOB gather index for padded positions

BF16NP = ml_dtypes.bfloat16


def _build_nc():
    nc = bacc.Bacc("TRN2", target_bir_lowering=False)

    u = nc.dram_tensor("u", [BL, NBLK, K, W * K], BF16, kind="ExternalInput")
    sc = nc.dram_tensor("sc", [BL, T, K, K], F32, kind="ExternalInput")
    # a0/afin travel transposed ([pair, 128]) so their DMAs are a few
    # 256B descriptors instead of hundreds of 4-byte ones whose
    # completion semaphores trickle for ~10us in the epilogue.
    a0t = nc.dram_tensor("a0t", [NPAIR, 128], BF16, kind="ExternalInput")
    id128 = nc.dram_tensor("id128", [128, 128], BF16, kind="ExternalInput")
    gidx = nc.dram_tensor("gidx", [128, G], I32, kind="ExternalInput")
    afint = nc.dram_tensor("afint", [4, NGRP * 128], BF16,
                           kind="ExternalOutput")
    goldv = nc.dram_tensor("goldv", [1, 1], F32, kind="ExternalOutput")

    with tile.TileContext(nc) as tc:
        with (
            tc.tile_pool(name="strips", bufs=3) as strips,
            tc.tile_pool(name="persist", bufs=1) as persist,
            tc.tile_pool(name="pers_psum", bufs=1, space="PSUM") as pers_psum,
        ):
            # ---- gold gather (gpsimd, off the scan's critical path) ---
            idxs = persist.tile([128, G], I32, tag="idxs", name="idxs")
            gath = persist.tile([128, G], F32, tag="gath", name="gath")
            goldsb = persist.tile([128, 1], F32, tag="goldsb", name="goldsb")
            nc.gpsimd.dma_start(out=idxs[:], in_=gidx[:])
            nc.gpsimd.memset(gath[:], 0.0)
            sc_flat = sc[:].rearrange(
                "b t kf (kto one) -> (b t kf kto) one", one=1
            )
            nc.gpsimd.indirect_dma_start(
                out=gath[:],
                out_offset=None,
                in_=sc_flat,
                in_offset=bass.IndirectOffsetOnAxis(ap=idxs[:], axis=0),
                bounds_check=BL * T * K * K - 1,
                oob_is_err=False,
            )
            # (the gold reduce is emitted AFTER the scan loop: the DVE
            # queue is in-order, and an early reduce would stall every
            # scan copy behind the slow indirect gather.)

            # ---- persistent state tiles -------------------------------
            # a_bufs[g][r]: [128, 4] bf16, packed: col jj holds the state
            # of row 2j in partitions 0-63 and row 2j+1 in 64-127.
            a_bufs = [
                [
                    persist.tile([128, len(GROUPS[g])], BF16,
                                 tag=f"a{g}_{r}", name=f"a{g}_{r}")
                    for r in range(3)
                ]
                for g in range(NGRP)
            ]
            # each PSUM tile padded to a full 2KB bank so the four
            # rotating tiles land in distinct banks (a shared bank
            # serializes group A's copy against group B's matmuls).
            ps_bufs = [
                [
                    pers_psum.tile([128, 512], F32,
                                   tag=f"ps{g}_{r}", name=f"ps{g}_{r}")
                    for r in range(2)
                ]
                for g in range(NGRP)
            ]

            # init: load transposed a0 + identity, transpose via PE
            a0t_sb = persist.tile([NPAIR, 128], BF16, tag="a0t", name="a0t")
            id_sb = persist.tile([128, 128], BF16, tag="id128", name="id128")
            ones = persist.tile([128, 1], F32, tag="ones", name="ones")
            afin_t = persist.tile([4, NGRP * 128], BF16, tag="afin_t",
                                  name="afin_t")
            goldf = persist.tile([1, 1], F32, tag="goldf", name="goldf")
            ps_aux = pers_psum.tile([128, 512], F32, tag="ps_aux",
                                    name="ps_aux")
            nc.sync.dma_start(out=a0t_sb[:], in_=a0t[:])
            nc.sync.dma_start(out=id_sb[:], in_=id128[:])
            nc.vector.memset(ones[:], 1.0)
            nc.tensor.matmul(
                out=ps_aux[:, 0:NPAIR],
                lhsT=a0t_sb[:].rearrange("p f -> p f"),
                rhs=id_sb[0:NPAIR, 0:NPAIR],
                start=True,
                stop=True,
            )
            off = 0
            for g in range(NGRP):
                ng = len(GROUPS[g])
                nc.vector.tensor_copy(
                    a_bufs[g][0][:], ps_aux[:, off : off + ng]
                )
                off += ng

            # ---- main scan --------------------------------------------
            # block 0 is split into a small starter segment (steps 1-3)
            # plus the rest, so the scan begins after a ~64KB load
            # instead of waiting for the full first block.
            segments = [(0, 0, 4), (0, 4, W)]
            segments += [(blk, 0, W) for blk in range(1, NBLK)]
            for blk, lo, hi in segments:
                width = (hi - lo) * K
                # alternate groups in strip emission order so neither
                # group's exp()s systematically finish first and push the
                # scheduler into serializing the groups.
                cur = [None] * NPAIR
                for j in (0, 3, 6, 1, 4, 7, 2, 5):
                    tag = (f"strip{j}" if hi - lo == W
                           else f"st{j}_{blk}_{lo}")
                    s = strips.tile([128, width], BF16, tag=tag)
                    # split descriptor generation across the sync and
                    # gpsimd queues so neither paces the block.
                    eng = nc.sync if j < 4 else nc.gpsimd
                    for h in range(2):
                        eng.dma_start(
                            out=s[64 * h : 64 * h + 64, :],
                            in_=u[2 * j + h, blk][:, lo * K : hi * K],
                        )
                    nc.scalar.activation(
                        s[:], s[:], mybir.ActivationFunctionType.Exp
                    )
                    cur[j] = s

                for tl in range(lo, hi):
                    t = blk * W + tl
                    if t == 0:
                        continue
                    for g in range(NGRP):
                        ps = ps_bufs[g][t % 2]
                        a_prev = a_bufs[g][(t - 1) % 3]
                        for jj, j in enumerate(GROUPS[g]):
                            sl = slice((tl - lo) * K, (tl - lo + 1) * K)
                            # even row: PE tile (0, 0); odd: (64, 64)
                            nc.tensor.matmul(
                                out=ps[0:64, jj : jj + 1],
                                lhsT=cur[j][0:64, sl],
                                rhs=a_prev[0:64, jj : jj + 1],
                                start=True,
                                stop=True,
                            )
                            nc.tensor.matmul(
                                out=ps[64:128, jj : jj + 1],
                                lhsT=cur[j][64:128, sl],
                                rhs=a_prev[64:128, jj : jj + 1],
                                start=True,
                                stop=True,
                            )
                        nc.vector.tensor_copy(
                            a_bufs[g][t % 3][:],
                            ps[:, 0 : len(GROUPS[g])],
                        )

            # ---- final state readout + gold reduce --------------------
            for g in range(NGRP):
                ng = len(GROUPS[g])
                nc.tensor.matmul(
                    out=ps_aux[0:ng, 128 * (g + 1) : 128 * (g + 1) + 128],
                    lhsT=a_bufs[g][(T - 1) % 3][:],
                    rhs=id_sb[:],
                    start=True,
                    stop=True,
                )
                nc.vector.tensor_copy(
                    afin_t[0:ng, 128 * g : 128 * g + 128],
                    ps_aux[0:ng, 128 * (g + 1) : 128 * (g + 1) + 128],
                )
            nc.sync.dma_start(out=afint[:], in_=afin_t[:])
            nc.vector.tensor_reduce(
                goldsb[:], gath[:],
                axis=mybir.AxisListType.XYZW, op=mybir.AluOpType.add,
            )
            nc.tensor.matmul(
                out=ps_aux[0:1, 100:101],
                lhsT=goldsb[:],
                rhs=ones[:],
                start=True,
                stop=True,
            )
            nc.vector.tensor_copy(goldf[:], ps_aux[0:1, 100:101])
            nc.sync.dma_start(out=goldv[:], in_=goldf[:])

    return nc


_NC_CACHE = None


def _get_nc():
    global _NC_CACHE
    if _NC_CACHE is None:
        _NC_CACHE = _build_nc()
        _NC_CACHE.finalize()
    return _NC_CACHE


def _make_in_maps(scores, targets, lengths):
    scores = np.asarray(scores, dtype=np.float32)
    targets = np.asarray(targets).astype(np.int64)
    lengths = np.asarray(lengths).astype(np.int64)

    # fold the per-step 2^-7 scale into the scores, then overwrite the
    # padded timesteps with the identity slab (exp == I exactly).
    shifted = scores - np.float32(LOG_C)
    pad_slab = np.full((K, K), PAD_OFFDIAG, dtype=np.float32)
    np.fill_diagonal(pad_slab, 0.0)
    for b in range(B):
        L = int(lengths[b])
        if L < T:
            shifted[b, L:] = pad_slab

    # a_0 = exp(sc'[b, 0, START, :]) per row (includes one 2^-7 factor)
    a0_all = np.exp(shifted[:, 0, START, :])  # [B, K] f64->f32
    a0_all = a0_all.astype(np.float32)

    in_maps = []
    for c in range(NCORES):
        sl = slice(c * BL, (c + 1) * BL)
        sh = shifted[sl]          # [BL, T, K, K]
        tg = targets[sl]          # [BL, T]
        ln = lengths[sl]          # [BL]

        # interleaved layout [b][blk][kf][tin][kto], bf16
        u = np.ascontiguousarray(
            sh.reshape(BL, NBLK, W, K, K).transpose(0, 1, 3, 2, 4)
        ).astype(BF16NP).reshape(BL, NBLK, K, W * K)

        # transposed a0: row j = [row 2j state (cols 0-63) ;
        #                         row 2j+1 state (cols 64-127)]
        a0t_arr = np.zeros((NPAIR, 128), dtype=BF16NP)
        for j in range(NPAIR):
            for h in range(2):
                bl = 2 * j + h
                a0t_arr[j, 64 * h : 64 * h + 64] = a0_all[c * BL + bl].astype(
                    BF16NP
                )

        # gold gather element indices into the raw f32 scores shard
        b_idx = np.arange(BL)[:, None]
        t_idx = np.arange(T)[None, :]
        flat = (b_idx * T + t_idx) * (K * K) + tg  # [BL, T]
        valid = t_idx < ln[:, None]
        flat = np.where(valid, flat, np.int64(SENTINEL))
        gidx = flat.reshape(128, G).astype(np.int32)

        im = {
            "u": u,
            "sc": np.ascontiguousarray(scores[sl]),
            "gidx": np.ascontiguousarray(gidx),
            "a0t": a0t_arr,
            "id128": np.eye(128, dtype=BF16NP),
        }
        in_maps.append(im)
    return in_maps, lengths


def _combine(results, lengths):
    # a_fin carries L factors of 2^-7 (one from a_0, one per real step),
    # so fs_b = log(a_fin[END]) + L_b * LOG_C; gold is gathered from the
    # raw (unshifted) scores.
    all_scores = 0.0
    gold_total = 0.0
    for c in range(NCORES):
        gold_total += float(results[c]["goldv"][0, 0])
        afint = results[c]["afint"].astype(np.float32)  # [4, NGRP*128]
        for bl in range(BL):
            L = int(lengths[c * BL + bl])
            pair = bl // 2
            g = next(i for i, grp in enumerate(GROUPS) if pair in grp)
            jj = GROUPS[g].index(pair)
            h = bl % 2
            a_end = float(afint[jj, 128 * g + 64 * h + END])
            all_scores += math.log(a_end) + L * LOG_C
    return np.float32((all_scores - gold_total) / B)


def kernel(scores, targets, lengths, trace=False):
    nc = _get_nc()
    in_maps, ln = _make_in_maps(scores, targets, lengths)
    res = run_bass_kernel_spmd(
        nc, in_maps, core_ids=list(range(NCORES)), trace=trace
    )
    out = _combine(res.results, ln)
    if trace:
        return out, res
    return out


# revision 39
# speedup vs baseline: 1.0170x; 1.0170x over previous
"""CRF loss kernel for Trainium2 (8 NeuronCores, data-parallel over batch).

Problem (hardcoded shapes): scores [B=128, T=256, K=64, K=64] f32,
targets [128, 256] int (flattened from_tag*K + to_tag), lengths [128] int.

loss = (sum_b fs[b, END] - gold) / B  where fs is the CRF forward
(log-domain) scan and gold is the gathered gold-path score.

Strategy (per core, 16 batch rows; ~242us HW vs the 845us f32 baseline):
  * Linear-domain forward scan with a constant per-step 2^-7 scale that
    is folded into the scores host-side (sc' = sc - 7*ln2), so the
    device step is a pure bf16 matmul + PSUM->SBUF cast:
        a_t = E'_t^T a_{t-1},   E'_t = exp(sc'_t)
  * Padded timesteps (t >= L_b) are overwritten host-side with an
    "identity slab" (0 on the diagonal, -100 off it), so exp() of them
    is exactly the identity matrix and the scan needs no per-row
    freezing, masking, or per-step state dumps: a_{T-1} == a_{L_b-1}
    automatically and only the final state is read back.
  * Host pre-interleaves the scores to [b][blk][kf][t_in_blk][kto]
    (bf16), so every DMA descriptor is a contiguous 2 KiB line; strip
    descriptor generation is split across the sync and gpsimd queues.
  * Each batch-row pair does two [64,64]x[64,1] bf16 matmuls per step:
    the even row at PE tile position (0,0), the odd row at (64,64)
    (derived from the partition offsets), so the next state lands
    PACKED in PSUM [128, 1] and ONE plain DVE copy per group per step
    casts it back to SBUF.  The 8 pairs form 3 groups (3/3/2) with
    independent PSUM-bank/state tiles; interleaving their strip/exp
    emission keeps the tile scheduler from serializing the chains, and
    the two spare group bursts hide each group's copy+semaphore chain.
  * gold: indirect DMA element-gather from the raw f32 scores (the
    2-byte gather path collapses per-partition indirection), masked by
    a huge sentinel index; the free-axis reduce + 128->1 matmul-reduce
    run after the scan (an early DVE reduce would head-block the
    in-order DVE queue behind the gather).
  * a0/afin travel transposed [pair, 128] and are transposed on-chip
    via PE identity matmuls: a [128, x] bf16 DRAM tensor would shatter
    into per-partition 4-byte packets whose completion semaphores
    trickle for ~10us in the epilogue.
  * Host finishes per row: fs_b = log(a_fin[END]) + L_b * 7*ln2.
"""

import math

import ml_dtypes
import numpy as np

import concourse.bacc as bacc
import concourse.bass as bass
import concourse.tile as tile
from concourse import mybir
from concourse.bass_utils import run_bass_kernel_spmd

F32 = mybir.dt.float32
BF16 = mybir.dt.bfloat16
I32 = mybir.dt.int32

B = 128
T = 256
K = 64
START = 62
END = 63
NCORES = 8
BL = B // NCORES          # 16 local batch rows per core
NPAIR = BL // 2           # 8
GROUPS = [[0, 1, 2], [3, 4, 5], [6, 7]]  # pipeline groups (pair ids)
NGRP = len(GROUPS)
W = 16                    # timesteps per strip
NBLK = T // W             # 16
G = BL * T // 128         # gold gather indices per partition (32)
LOG_C = 7.0 * math.log(2.0)  # per-step scale 2^-7, folded into scores
PAD_OFFDIAG = -100.0      # exp() == 0 in bf16
SENTINEL = 0x7FFFFF00     # OOB gather index for padded positions

BF16NP = ml_dtypes.bfloat16


def _build_nc():
    nc = bacc.Bacc("TRN2", target_bir_lowering=False)

    u = nc.dram_tensor("u", [BL, NBLK, K, W * K], BF16, kind="ExternalInput")
    sc = nc.dram_tensor("sc", [BL, T, K, K], F32, kind="ExternalInput")
    # a0/afin travel transposed ([pair, 128]) so their DMAs are a few
    # 256B descriptors instead of hundreds of 4-byte ones whose
    # completion semaphores trickle for ~10us in the epilogue.
    a0t = nc.dram_tensor("a0t", [NPAIR, 128], BF16, kind="ExternalInput")
    id128 = nc.dram_tensor("id128", [128, 128], BF16, kind="ExternalInput")
    gidx = nc.dram_tensor("gidx", [128, G], I32, kind="ExternalInput")
    afint = nc.dram_tensor("afint", [4, NGRP * 128], BF16,
                           kind="ExternalOutput")
    goldv = nc.dram_tensor("goldv", [1, 1], F32, kind="ExternalOutput")

    with tile.TileContext(nc) as tc:
        with (
            tc.tile_pool(name="strips", bufs=2) as strips,
            tc.tile_pool(name="persist", bufs=1) as persist,
            tc.tile_pool(name="pers_psum", bufs=1, space="PSUM") as pers_psum,
        ):
            # ---- gold gather (gpsimd, off the scan's critical path) ---
            idxs = persist.tile([128, G], I32, tag="idxs", name="idxs")
            gath = persist.tile([128, G], F32, tag="gath", name="gath")
            goldsb = persist.tile([128, 1], F32, tag="goldsb", name="goldsb")
            nc.gpsimd.dma_start(out=idxs[:], in_=gidx[:])
            nc.gpsimd.memset(gath[:], 0.0)
            sc_flat = sc[:].rearrange(
                "b t kf (kto one) -> (b t kf kto) one", one=1
            )
            nc.gpsimd.indirect_dma_start(
                out=gath[:],
                out_offset=None,
                in_=sc_flat,
                in_offset=bass.IndirectOffsetOnAxis(ap=idxs[:], axis=0),
                bounds_check=BL * T * K * K - 1,
                oob_is_err=False,
            )
            # (the gold reduce is emitted AFTER the scan loop: the DVE
            # queue is in-order, and an early reduce would stall every
            # scan copy behind the slow indirect gather.)

            # ---- persistent state tiles -------------------------------
            # a_bufs[g][r]: [128, 4] bf16, packed: col jj holds the state
            # of row 2j in partitions 0-63 and row 2j+1 in 64-127.
            a_bufs = [
                [
                    persist.tile([128, len(GROUPS[g])], BF16,
                                 tag=f"a{g}_{r}", name=f"a{g}_{r}")
                    for r in range(3)
                ]
                for g in range(NGRP)
            ]
            # each PSUM tile padded to a full 2KB bank so the four
            # rotating tiles land in distinct banks (a shared bank
            # serializes group A's copy against group B's matmuls).
            ps_bufs = [
                [
                    pers_psum.tile([128, 512], F32,
                                   tag=f"ps{g}_{r}", name=f"ps{g}_{r}")
                    for r in range(2)
                ]
                for g in range(NGRP)
            ]

            # init: load transposed a0 + identity, transpose via PE
            a0t_sb = persist.tile([NPAIR, 128], BF16, tag="a0t", name="a0t")
            id_sb = persist.tile([128, 128], BF16, tag="id128", name="id128")
            ones = persist.tile([128, 1], F32, tag="ones", name="ones")
            afin_t = persist.tile([4, NGRP * 128], BF16, tag="afin_t",
                                  name="afin_t")
            goldf = persist.tile([1, 1], F32, tag="goldf", name="goldf")
            ps_aux = pers_psum.tile([128, 512], F32, tag="ps_aux",
                                    name="ps_aux")
            nc.sync.dma_start(out=a0t_sb[:], in_=a0t[:])
            nc.sync.dma_start(out=id_sb[:], in_=id128[:])
            nc.vector.memset(ones[:], 1.0)
            nc.tensor.matmul(
                out=ps_aux[:, 0:NPAIR],
                lhsT=a0t_sb[:].rearrange("p f -> p f"),
                rhs=id_sb[0:NPAIR, 0:NPAIR],
                start=True,
                stop=True,
            )
            off = 0
            for g in range(NGRP):
                ng = len(GROUPS[g])
                nc.vector.tensor_copy(
                    a_bufs[g][0][:], ps_aux[:, off : off + ng]
                )
                off += ng

            # ---- main scan --------------------------------------------
            # block 0 is split into a small starter segment (steps 1-3)
            # plus the rest, so the scan begins after a ~64KB load
            # instead of waiting for the full first block.
            segments = [(0, 0, 4), (0, 4, W)]
            segments += [(blk, 0, W) for blk in range(1, NBLK)]
            for blk, lo, hi in segments:
                width = (hi - lo) * K
                # alternate groups in strip emission order so neither
                # group's exp()s systematically finish first and push the
                # scheduler into serializing the groups.
                cur = [None] * NPAIR
                for j in (0, 3, 6, 1, 4, 7, 2, 5):
                    tag = (f"strip{j}" if hi - lo == W
                           else f"st{j}_{blk}_{lo}")
                    s = strips.tile([128, width], BF16, tag=tag)
                    # split descriptor generation across the sync and
                    # gpsimd queues so neither paces the block.
                    eng = nc.sync if j < 4 else nc.gpsimd
                    for h in range(2):
                        eng.dma_start(
                            out=s[64 * h : 64 * h + 64, :],
                            in_=u[2 * j + h, blk][:, lo * K : hi * K],
                        )
                    nc.scalar.activation(
                        s[:], s[:], mybir.ActivationFunctionType.Exp
                    )
                    cur[j] = s

                for tl in range(lo, hi):
                    t = blk * W + tl
                    if t == 0:
                        continue
                    for g in range(NGRP):
                        ps = ps_bufs[g][t % 2]
                        a_prev = a_bufs[g][(t - 1) % 3]
                        for jj, j in enumerate(GROUPS[g]):
                            sl = slice((tl - lo) * K, (tl - lo + 1) * K)
                            # even row: PE tile (0, 0); odd: (64, 64)
                            nc.tensor.matmul(
                                out=ps[0:64, jj : jj + 1],
                                lhsT=cur[j][0:64, sl],
                                rhs=a_prev[0:64, jj : jj + 1],
                                start=True,
                                stop=True,
                            )
                            nc.tensor.matmul(
                                out=ps[64:128, jj : jj + 1],
                                lhsT=cur[j][64:128, sl],
                                rhs=a_prev[64:128, jj : jj + 1],
                                start=True,
                                stop=True,
                            )
                        nc.vector.tensor_copy(
                            a_bufs[g][t % 3][:],
                            ps[:, 0 : len(GROUPS[g])],
                        )

            # ---- final state readout + gold reduce --------------------
            for g in range(NGRP):
                ng = len(GROUPS[g])
                nc.tensor.matmul(
                    out=ps_aux[0:ng, 128 * (g + 1) : 128 * (g + 1) + 128],
                    lhsT=a_bufs[g][(T - 1) % 3][:],
                    rhs=id_sb[:],
                    start=True,
                    stop=True,
                )
                nc.vector.tensor_copy(
                    afin_t[0:ng, 128 * g : 128 * g + 128],
                    ps_aux[0:ng, 128 * (g + 1) : 128 * (g + 1) + 128],
                )
            nc.sync.dma_start(out=afint[:], in_=afin_t[:])
            nc.vector.tensor_reduce(
                goldsb[:], gath[:],
                axis=mybir.AxisListType.XYZW, op=mybir.AluOpType.add,
            )
            nc.tensor.matmul(
                out=ps_aux[0:1, 100:101],
                lhsT=goldsb[:],
                rhs=ones[:],
                start=True,
                stop=True,
            )
            nc.vector.tensor_copy(goldf[:], ps_aux[0:1, 100:101])
            nc.sync.dma_start(out=goldv[:], in_=goldf[:])

    return nc


_NC_CACHE = None


def _get_nc():
    global _NC_CACHE
    if _NC_CACHE is None:
        _NC_CACHE = _build_nc()
        _NC_CACHE.finalize()
    return _NC_CACHE


def _make_in_maps(scores, targets, lengths):
    scores = np.asarray(scores, dtype=np.float32)
    targets = np.asarray(targets).astype(np.int64)
    lengths = np.asarray(lengths).astype(np.int64)

    # fold the per-step 2^-7 scale into the scores, then overwrite the
    # padded timesteps with the identity slab (exp == I exactly).
    shifted = scores - np.float32(LOG_C)
    pad_slab = np.full((K, K), PAD_OFFDIAG, dtype=np.float32)
    np.fill_diagonal(pad_slab, 0.0)
    for b in range(B):
        L = int(lengths[b])
        if L < T:
            shifted[b, L:] = pad_slab

    # a_0 = exp(sc'[b, 0, START, :]) per row (includes one 2^-7 factor)
    a0_all = np.exp(shifted[:, 0, START, :])  # [B, K] f64->f32
    a0_all = a0_all.astype(np.float32)

    in_maps = []
    for c in range(NCORES):
        sl = slice(c * BL, (c + 1) * BL)
        sh = shifted[sl]          # [BL, T, K, K]
        tg = targets[sl]          # [BL, T]
        ln = lengths[sl]          # [BL]

        # interleaved layout [b][blk][kf][tin][kto], bf16
        u = np.ascontiguousarray(
            sh.reshape(BL, NBLK, W, K, K).transpose(0, 1, 3, 2, 4)
        ).astype(BF16NP).reshape(BL, NBLK, K, W * K)

        # transposed a0: row j = [row 2j state (cols 0-63) ;
        #                         row 2j+1 state (cols 64-127)]
        a0t_arr = np.zeros((NPAIR, 128), dtype=BF16NP)
        for j in range(NPAIR):
            for h in range(2):
                bl = 2 * j + h
                a0t_arr[j, 64 * h : 64 * h + 64] = a0_all[c * BL + bl].astype(
                    BF16NP
                )

        # gold gather element indices into the raw f32 scores shard
        b_idx = np.arange(BL)[:, None]
        t_idx = np.arange(T)[None, :]
        flat = (b_idx * T + t_idx) * (K * K) + tg  # [BL, T]
        valid = t_idx < ln[:, None]
        flat = np.where(valid, flat, np.int64(SENTINEL))
        gidx = flat.reshape(128, G).astype(np.int32)

        im = {
            "u": u,
            "sc": np.ascontiguousarray(scores[sl]),
            "gidx": np.ascontiguousarray(gidx),
            "a0t": a0t_arr,
            "id128": np.eye(128, dtype=BF16NP),
        }
        in_maps.append(im)
    return in_maps, lengths


def _combine(results, lengths):
    # a_fin carries L factors of 2^-7 (one from a_0, one per real step),
    # so fs_b = log(a_fin[END]) + L_b * LOG_C; gold is gathered from the
    # raw (unshifted) scores.
    all_scores = 0.0
    gold_total = 0.0
    for c in range(NCORES):
        gold_total += float(results[c]["goldv"][0, 0])
        afint = results[c]["afint"].astype(np.float32)  # [4, NGRP*128]
        for bl in range(BL):
            L = int(lengths[c * BL + bl])
            pair = bl // 2
            g = next(i for i, grp in enumerate(GROUPS) if pair in grp)
            jj = GROUPS[g].index(pair)
            h = bl % 2
            a_end = float(afint[jj, 128 * g + 64 * h + END])
            all_scores += math.log(a_end) + L * LOG_C
    return np.float32((all_scores - gold_total) / B)


def kernel(scores, targets, lengths, trace=False):
    nc = _get_nc()
    in_maps, ln = _make_in_maps(scores, targets, lengths)
    res = run_bass_kernel_spmd(
        nc, in_maps, core_ids=list(range(NCORES)), trace=trace
    )
    out = _combine(res.results, ln)
    if trace:
        return out, res
    return out
